# revision 43
# baseline (speedup 1.0000x reference)
"""APPNP GNN kernel for 8 TRN2 NeuronCores — raw Bass engine programs.

Per core (dest-stationary sharding):
  GEMM:  h = relu(x@W1'), z0 = h@W2 + b2  (data-parallel over its nodes)
  K x propagation:
     dma_gather z rows (f32 256B rows, int16 idx in overlapping 32K windows)
     DVE: scale by edge norm -> bf16 messages
     PE:  identity-matmul accumulate into PSUM per 128-dest block
     DVE: z_new = 0.9*(agg + selfloop*z) + 0.1*z0
     AllGather bf16 shards -> replicated z_full
  log_softmax epilogue; host unpermutes rows.
"""

import numpy as np
import ml_dtypes

import concourse.bass as bass
import concourse.bacc as bacc
import concourse.mybir as mybir
from concourse.library_config import mlp as mlp_lib

F32 = mybir.dt.float32
BF16 = mybir.dt.bfloat16
I16 = mybir.dt.int16

NCORES = 8
P = 128
ALPHA = 0.1

CALL_TILES = 8     # tiles per dma_gather call (1024 idxs; safe at depth 2)
GDEPTH = 4
SP_FLAG = True
NQ = 4
GRP = 2            # dest blocks per group
WINROWS = 32768
KITER_DEFAULT = 10
GINC = 16   # sem increment per dma_gather on HW (sim models 32)
NIB_DOC = """idx/norm stream is prefetched 4-deep by the ACT engine; a dummy
flush gather per queue closes each iteration so completion sems can't lead
the data writes observed by DVE."""
DBG_NO_SOFTMAX = False
DBG_NO_AG = False
DBG_NO_GATHER = False
DBG_NO_LEVELOPT = False


def _sigmoid(v):
    return 1.0 / (1.0 + np.exp(-v.astype(np.float64)))


# ---------------------------------------------------------------------------
# host preprocessing
# ---------------------------------------------------------------------------

def _preprocess(x, edge_index, edge_weight_train, x_weight, W1, b1, W2, b2, kiter):
    N, F = x.shape
    D = W1.shape[1]
    C = W2.shape[1]
    row = np.asarray(edge_index[0], np.int64)
    col = np.asarray(edge_index[1], np.int64)

    ew = np.asarray(edge_weight_train, np.float32)
    mask = np.abs(ew) > 0
    w = np.where(mask, _sigmoid(ew), 0.0).astype(np.float32)
    deg = (np.bincount(col, weights=w.astype(np.float64), minlength=N) + 1.0).astype(np.float32)
    dinv = np.where(deg > 0, 1.0 / np.sqrt(np.maximum(deg, 1e-12)), 0.0).astype(np.float32)
    enorm = (dinv[row] * w * dinv[col]).astype(np.float32)
    sdiag = (dinv * dinv).astype(np.float32)

    xw = (_sigmoid(np.asarray(x_weight, np.float32))
          * (np.abs(x_weight) > 0)).astype(np.float32)
    W1p = xw[:, None] * np.asarray(W1, np.float32)

    assert N % NCORES == 0
    SH = N // NCORES
    NB = (SH + P - 1) // P
    SHP = NB * P
    NEFF = SHP * NCORES
    cnt = np.bincount(col, minlength=N)
    order = np.argsort(-cnt, kind="stable")
    pat = np.concatenate([np.arange(NCORES), np.arange(NCORES)[::-1]])
    core_of_rank = pat[np.arange(N) % (2 * NCORES)]
    if np.bincount(core_of_rank, minlength=NCORES).max() != SH:
        core_of_rank = np.repeat(np.arange(NCORES), SH)
    pos = np.empty(N, np.int64)
    core_of_node = np.empty(N, np.int64)
    for c in range(NCORES):
        nodes_c = order[core_of_rank == c]
        pos[nodes_c] = c * SHP + np.arange(len(nodes_c))
        core_of_node[nodes_c] = c

    if NEFF <= WINROWS:
        bases = np.array([0], np.int64)
    else:
        nw = 4
        bases = np.array([int(round(k * (NEFF - WINROWS) / (nw - 1)))
                          for k in range(nw)], np.int64)
    NW = len(bases)
    ND = NEFF

    def _window_fill(pos_a):
        src_pos = pos_a[row]
        dloc = pos_a[col]
        wlo = np.searchsorted(bases + WINROWS - 1, src_pos, side="left")
        whi = np.searchsorted(bases, src_pos, side="right") - 1
        assert (wlo <= whi).all()
        f = np.zeros((NW, ND), np.int64)
        flex = np.zeros((max(NW - 1, 1), ND), np.int64)
        forced_m = wlo == whi
        for wi in range(NW):
            np.add.at(f[wi], dloc[forced_m & (wlo == wi)], 1)
        for wi in range(NW - 1):
            np.add.at(flex[wi], dloc[(~forced_m) & (wlo == wi)], 1)
        degp = np.bincount(dloc, minlength=ND)
        tgt = (degp + NW - 1) // NW
        takes = np.zeros((max(NW - 1, 1), ND), np.int64)
        cw = np.zeros((NW, ND), np.int64)
        carry = np.zeros(ND, np.int64)
        for wi in range(NW):
            base_c = f[wi] + carry
            if wi < NW - 1:
                take = np.clip(tgt - base_c, 0, flex[wi])
                takes[wi] = take
                carry = flex[wi] - take
                cw[wi] = base_c + take
            else:
                cw[wi] = base_c
        return src_pos, dloc, wlo, whi, forced_m, f, flex, takes, cw, degp

    # profile-clustered block assignment: 2 refinement passes grouping dests
    # with similar (deg, per-window count) profiles into the same block to
    # shrink max-over-partitions padding (~15% less gather traffic).
    if NW > 1:
        for _it in range(2):
            _, _, _, _, _, _, _, _, cw_i, _ = _window_fill(pos)
            for c in range(NCORES):
                nodes_c = np.nonzero(core_of_node == c)[0]
                key = (cnt[nodes_c] * 32768
                       + cw_i[0][pos[nodes_c]] * 1024
                       + cw_i[1][pos[nodes_c]] * 32
                       + cw_i[2][pos[nodes_c]])
                o = np.argsort(-key, kind="stable")
                pos[nodes_c[o]] = c * SHP + np.arange(len(nodes_c))

    perm = np.full(NEFF, -1, np.int64)
    perm[pos] = np.arange(N)
    src_pos, dloc, wlo, whi, forced_m, f, flex, takes, cw, degp = _window_fill(pos)
    dst_pos = dloc

    # Per-block level optimization of the flex window assignment: for each
    # dest block, greedily lower the per-window level vector L (shared across
    # cores) subject to per-node chain feasibility, then re-derive takes/cw.
    # Cuts ~4% of gather padding vs the per-node tgt waterfill.
    if NW > 1 and not DBG_NO_LEVELOPT:
        b_of_all = (np.arange(NEFF) % SHP) // P

        def _chain(fb, flb, L):
            M = fb.shape[1]
            carry = np.zeros(M, np.int64)
            tk = np.zeros((NW - 1, M), np.int64)
            for w in range(NW):
                mustv = fb[w] + carry
                if (mustv > L[w]).any():
                    return None
                if w < NW - 1:
                    t = np.minimum(flb[w], L[w] - mustv)
                    tk[w] = t
                    carry = flb[w] - t
            return tk

        new_takes = np.zeros_like(takes)
        for b in range(NB):
            sel = b_of_all == b
            fb = f[:, sel]
            flb = flex[:, sel]
            cands = []
            for Linit in (cw[:, sel].max(axis=1),):
                L = Linit.astype(np.int64).copy()
                assert _chain(fb, flb, L) is not None
                improved = True
                while improved:
                    improved = False
                    for w in np.argsort(-L):
                        if L[w] == 0:
                            continue
                        L[w] -= 1
                        if _chain(fb, flb, L) is None:
                            L[w] += 1
                        else:
                            improved = True
                cands.append((int(L.sum()), L))
            L = min(cands)[1]
            new_takes[:, sel] = _chain(fb, flb, L)
        takes = new_takes
        carry = np.zeros(NEFF, np.int64)
        for wi in range(NW):
            base_c = f[wi] + carry
            if wi < NW - 1:
                carry = flex[wi] - takes[wi]
                cw[wi] = base_c + takes[wi]
            else:
                cw[wi] = base_c

    cw3 = cw.reshape(NW, NCORES, NB, P)
    T = cw3.max(axis=(1, 3)).T          # [NB, NW] shared tile table
    # every block must own one full 8-tile chunk (start=True zeroes the bank)
    for b in range(NB):
        wmax = int(np.argmax(T[b]))
        if T[b, wmax] < 8:
            T[b, wmax] = 8

    ngroups = (NB + GRP - 1) // GRP
    tile_off = np.zeros((NB, NW), np.int64)
    grp_tile_off = np.zeros(ngroups + 1, np.int64)
    tcol = 0
    for g in range(ngroups):
        grp_tile_off[g] = tcol
        for wi in range(NW):
            for b in range(g * GRP, min((g + 1) * GRP, NB)):
                tile_off[b, wi] = tcol
                tcol += T[b, wi]
    grp_tile_off[ngroups] = tcol
    T_tot = int(tcol)

    calls = []
    qn = 0
    grp_call_end = np.zeros(ngroups, np.int64)
    cumq = np.zeros((ngroups, NQ), np.int64)
    qcount = np.zeros(NQ, np.int64)
    for g in range(ngroups):
        for wi in range(NW):
            lo = None
            n = 0
            for b in range(g * GRP, min((g + 1) * GRP, NB)):
                if T[b, wi]:
                    if lo is None:
                        lo = tile_off[b, wi]
                    n += T[b, wi]
            t = 0
            while t < n:
                nt = min(CALL_TILES, n - t)
                calls.append((qn % NQ, int(nt), int(lo + t), int(bases[wi]), g))
                qcount[qn % NQ] += 1
                qn += 1
                t += nt
        grp_call_end[g] = len(calls)
        cumq[g] = qcount

    blk_chunks = []
    for b in range(NB):
        ch = []
        worder = np.argsort(-T[b], kind="stable")
        for wi in worder:
            t = 0
            while t < T[b, wi]:
                nt = min(8, T[b, wi] - t)
                ch.append((int(tile_off[b, wi] + t), int(nt)))
                t += nt
        assert not ch or ch[0][1] == 8
        blk_chunks.append(ch)

    # edge -> (window, rank-in-window) consistent with the waterfill
    win = np.where(forced_m, wlo, 0)
    flex_m = ~forced_m
    fkey = np.where(flex_m, wlo, -1)
    okey = dloc * (2 * NW) + np.where(flex_m, NW + fkey, win)
    eord = np.argsort(okey, kind="stable")
    sk = okey[eord]
    grp_start = np.r_[0, np.nonzero(np.diff(sk))[0] + 1]
    starts_full = np.zeros(len(sk), np.int64)
    starts_full[grp_start] = grp_start
    np.maximum.accumulate(starts_full, out=starts_full)
    rank_sorted = np.arange(len(sk)) - starts_full
    rank = np.empty(len(sk), np.int64)
    rank[eord] = rank_sorted

    ewin = np.where(forced_m, wlo, -1)
    if NW > 1:
        tk = takes[np.clip(fkey, 0, NW - 2), dloc]
        ewin = np.where(flex_m, np.where(rank < tk, wlo, wlo + 1), ewin)
    r_in_w = rank.copy()
    if NW > 1:
        carry_prev = np.zeros((NW, ND), np.int64)
        for wi in range(1, NW):
            carry_prev[wi] = flex[wi - 1] - takes[wi - 1]
        up = flex_m & (ewin == fkey)
        dn = flex_m & (ewin == fkey + 1)
        r_in_w[up] = f[ewin[up], dloc[up]] + carry_prev[ewin[up], dloc[up]] + rank[up]
        r_in_w[dn] = f[ewin[dn], dloc[dn]] + (rank[dn] - takes[np.clip(fkey[dn], 0, NW - 2), dloc[dn]])
    assert (r_in_w < cw[ewin, dloc]).all()

    b_of = (dloc % SHP) // P
    p_of = dloc % P
    core_of = dloc // SHP
    tcol_of = tile_off[b_of, ewin] + r_in_w
    idx16 = (src_pos - bases[ewin]).astype(np.int64)
    assert (idx16 >= 0).all() and (idx16 < WINROWS).all()

    core_data = []
    xf = np.asarray(x, np.float32)
    for c in range(NCORES):
        sel = core_of == c
        ig = np.zeros((T_tot, P), np.int16)
        ng = np.zeros((P, T_tot), np.float32)
        ig[tcol_of[sel], p_of[sel]] = idx16[sel].astype(np.int16)
        ng[p_of[sel], tcol_of[sel]] = enorm[sel]
        idx_cols = np.ascontiguousarray(ig.reshape(-1).reshape(-1, 16).T)
        idx_all = np.ascontiguousarray(np.tile(idx_cols, (8, 1)))
        s_sb = np.zeros((P, NB), np.float32)
        rows = np.arange(SHP)
        nodes = perm[c * SHP + rows]
        valid = nodes >= 0
        s_sb[rows[valid] % P, rows[valid] // P] = sdiag[nodes[valid]]
        xsh = np.zeros((SHP, F), np.float32)
        xsh[rows[valid]] = xf[nodes[valid]]
        xT = np.ascontiguousarray(xsh.T).astype(ml_dtypes.bfloat16)
        core_data.append(dict(idx_all=idx_all, norm_all=np.ascontiguousarray(ng),
                              s_sb=s_sb, xT=xT))

    return dict(
        N=N, F=F, D=D, C=C, SH=SH, SHP=SHP, NB=NB, NEFF=NEFF, NW=NW,
        ngroups=ngroups, T_tot=T_tot, kiter=kiter,
        bases=bases, calls=calls, cumq=cumq, grp_call_end=grp_call_end,
        grp_tile_off=grp_tile_off, blk_chunks=blk_chunks, T=T,
        perm=perm, pos=pos, core_data=core_data,
        W1p=W1p.astype(ml_dtypes.bfloat16),
        W2=np.asarray(W2, np.float32).astype(ml_dtypes.bfloat16),
    )


# ---------------------------------------------------------------------------
# device program
# ---------------------------------------------------------------------------

def _build(meta):
    F, D, C, SHP, NB, NEFF = (meta[k] for k in ("F", "D", "C", "SHP", "NB", "NEFF"))
    K = meta["kiter"]
    NG = meta["ngroups"]
    calls = meta["calls"]
    cumq = meta["cumq"]
    gto = meta["grp_tile_off"]
    blk_chunks = meta["blk_chunks"]
    T_tot = meta["T_tot"]
    KC = F // P
    MH = D // P
    NT = GRP * P
    ntile = NG
    Tg_max = int(max(gto[g + 1] - gto[g] for g in range(NG)))
    qseq = []
    qc = [0] * NQ
    first_call_of_grp = {}
    for ci, (q, nt, tcol, wb, g) in enumerate(calls):
        if g not in first_call_of_grp:
            first_call_of_grp[g] = ci
        qseq.append(qc[q])
        qc[q] += 1
    # one dummy flush gather per queue closes each iteration: its completion
    # sem (in-order ring) guarantees every real call's SBUF writes landed.
    CPQ = [c + 1 for c in qc]
    last_call = calls[-1]

    nc = bacc.Bacc("TRN2", target_bir_lowering=False, debug=False,
                   num_devices=NCORES, num_swdge_queues=NQ)

    xT_d = nc.dram_tensor("xT", [F, SHP], BF16, kind="ExternalInput")
    w1_d = nc.dram_tensor("W1p", [F, D], BF16, kind="ExternalInput")
    w2_d = nc.dram_tensor("W2", [D, C], BF16, kind="ExternalInput")
    b1_d = nc.dram_tensor("b1p", [P, MH], F32, kind="ExternalInput")
    b2_d = nc.dram_tensor("b2b", [P, C], F32, kind="ExternalInput")
    idf32_d = nc.dram_tensor("idf32", [P, P], F32, kind="ExternalInput")
    idbf_d = nc.dram_tensor("idbf", [P, P], BF16, kind="ExternalInput")
    sdg_d = nc.dram_tensor("sdg", [P, NB], F32, kind="ExternalInput")
    idx_d = nc.dram_tensor("idxs", [P, T_tot * 8], I16, kind="ExternalInput")
    nrm_d = nc.dram_tensor("nrms", [P, T_tot], F32, kind="ExternalInput")
    out_d = nc.dram_tensor("out", [SHP, C], F32, kind="ExternalOutput")

    zfull = [nc.dram_tensor(f"zfull{i}", [NEFF, C], F32, addr_space="Shared")
             for i in range(2)]
    zsh_d = nc.dram_tensor("zsh", [SHP, C], F32)
    z0s_d = nc.dram_tensor("z0s", [SHP, C], F32)

    A = nc.alloc_sbuf_tensor
    w1_sb = A("w1_sb", [P, KC, D], BF16)
    w2_sb = A("w2_sb", [P, MH, C], BF16)
    b1_sb = A("b1_sb", [P, MH], F32)
    b2_sb = A("b2_sb", [P, C], F32)
    idf32 = A("idf32_sb", [P, P], F32)
    idbf = A("idbf_sb", [P, P], BF16)
    sdg_sb = A("sdg_sb", [P, NB], F32)
    xt_sb = [A(f"xt{i}", [P, KC, NT], BF16) for i in range(2)]
    ht_sb = [A(f"ht{i}", [P, MH, NT], BF16) for i in range(2)]
    zf_sb = [A(f"zf{i}", [P, GRP * C], F32) for i in range(2)]
    z0o_sb = [A(f"z0o{i}", [P, GRP * C], F32) for i in range(2)]
    gbuf = [A(f"gbuf{i}", [P, Tg_max * C], F32) for i in range(2)]
    mbuf = A("mbuf", [P, Tg_max * C], BF16)
    # idx/norm tables resident in SBUF: loaded once at boot, reused all K
    # iterations (saves ~12MB/iter of HBM re-DMA + per-group sync).
    # 4-deep idx/norm stream buffers, prefetched by the ACT engine so the
    # gather stream never queues behind SP's z-traffic waits.
    NIB = 4
    idxb = [A(f"idxb{i}", [P, Tg_max * 8], I16) for i in range(NIB)]
    nrmb = [A(f"nrmb{i}", [P, Tg_max], F32) for i in range(NIB)]
    scr_sb = A("scr_sb", [P, C], F32)
    zin_sb = [A(f"zin{i}", [P, GRP * C], F32) for i in range(2)]
    z0in_sb = [A(f"z0in{i}", [P, GRP * C], F32) for i in range(2)]
    znew_sb = [A(f"znew{i}", [P, GRP * C], F32) for i in range(2)]
    agg_sb = A("agg_sb", [P, GRP * C], F32)
    tmp_sb = A("tmp_sb", [P, GRP * C], F32)
    nm_sb = A("nm_sb", [P, GRP], F32)
    se_sb = A("se_sb", [P, GRP], F32)
    lse_sb = A("lse_sb", [P, GRP], F32)
    esc_sb = A("esc_sb", [P, GRP * C], F32)
    sm_sb = [A(f"sm{i}", [P, GRP * C], F32) for i in range(2)]

    psum = nc.alloc_psum_tensor("psumall", [P, 4096], F32)

    S = nc.alloc_semaphore
    s_boot = S("s_boot"); s_idx = S("s_idx"); s_scale = S("s_scale")
    s_mm = S("s_mm"); s_epi = S("s_epi"); s_ms = S("s_ms")
    s_zin = S("s_zin"); s_znw = S("s_znw"); s_z0w = S("s_z0w")
    s_cc = S("s_cc"); s_x = S("s_x")
    s_pe1 = S("s_pe1"); s_pe2 = S("s_pe2"); s_pe3 = S("s_pe3")
    s_act1 = S("s_act1"); s_act2 = S("s_act2"); s_act3 = S("s_act3")
    s_fz = S("s_fz"); s_sm1 = S("s_sm1"); s_sm2 = S("s_sm2")
    s_sm = S("s_sm"); s_out = S("s_out"); s_dve = S("s_dve"); s_dbg = S("s_dbg")
    sg = [S(f"sg{q}") for q in range(NQ)]

    NBOOT = 7

    def blocks_thru(g):
        return min((g + 1) * GRP, NB)

    def tw(t):
        return min(NT, SHP - t * NT)

    with nc.Block() as block:

        # ---------------- SP ----------------
        @block.sync
        def _(sp):
            sp.dma_start(w1_sb[:], w1_d[:].rearrange("(kc p) m -> p kc m", p=P)).then_inc(s_boot, 16)
            sp.dma_start(w2_sb[:], w2_d[:].rearrange("(mh p) m -> p mh m", p=P)).then_inc(s_boot, 16)
            sp.dma_start(b1_sb[:], b1_d[:]).then_inc(s_boot, 16)
            sp.dma_start(b2_sb[:], b2_d[:]).then_inc(s_boot, 16)
            sp.dma_start(idf32[:], idf32_d[:]).then_inc(s_boot, 16)
            sp.dma_start(idbf[:], idbf_d[:]).then_inc(s_boot, 16)
            sp.dma_start(sdg_sb[:], sdg_d[:]).then_inc(s_boot, 16)

            for t in range(ntile + 2):
                if t < ntile:
                    w = tw(t)
                    if t >= 2:
                        sp.wait_ge(s_pe1, t - 1)
                    sp.dma_start(
                        xt_sb[t % 2][:, :, :w],
                        xT_d[:].rearrange("(kc p) n -> p kc n", p=P)[:, :, t * NT:t * NT + w],
                    ).then_inc(s_x, 16)
                if t >= 2:
                    u = t - 2
                    w = tw(u)
                    sp.wait_ge(s_fz, u + 1)
                    sp.dma_start(
                        zsh_d[u * NT:u * NT + w, :].rearrange("(j p) c -> p j c", p=P),
                        zf_sb[u % 2][:, :(w // P) * C].rearrange("p (j c) -> p j c", c=C),
                    ).then_inc(s_znw, 16)
                    sp.wait_ge(s_act3, u + 1)
                    sp.dma_start(
                        z0s_d[u * NT:u * NT + w, :].rearrange("(j p) c -> p j c", p=P),
                        z0o_sb[u % 2][:, :(w // P) * C].rearrange("p (j c) -> p j c", c=C),
                    ).then_inc(s_z0w, 16)

            for k in range(K):
                for g in range(NG):
                    gb = k * NG + g
                    nb_g = blocks_thru(g) - g * GRP
                    rows = nb_g * P
                    if gb >= 2:
                        sp.wait_ge(s_epi, gb - 1)
                    sp.wait_ge(s_znw, 16 * (k * NG + g + 1))
                    sp.dma_start(
                        zin_sb[gb % 2][:, :nb_g * C].rearrange("p (j c) -> p j c", c=C),
                        zsh_d[g * NT:g * NT + rows, :].rearrange("(j p) c -> p j c", p=P),
                    ).then_inc(s_zin, 16)
                    sp.wait_ge(s_z0w, 16 * min(g + 1, NG))
                    sp.dma_start(
                        z0in_sb[gb % 2][:, :nb_g * C].rearrange("p (j c) -> p j c", c=C),
                        z0s_d[g * NT:g * NT + rows, :].rearrange("(j p) c -> p j c", p=P),
                    ).then_inc(s_zin, 16)
                    sp.wait_ge(s_epi, gb + 1)
                    sp.dma_start(
                        zsh_d[g * NT:g * NT + rows, :].rearrange("(j p) c -> p j c", p=P),
                        znew_sb[gb % 2][:, :nb_g * C].rearrange("p (j c) -> p j c", c=C),
                    ).then_inc(s_znw, 16)

            if DBG_NO_SOFTMAX:
                sp.wait_ge(s_epi, K * NG)
                sp.wait_ge(s_znw, 16 * (K + 1) * NG)
                sp.dma_start(out_d[:], zsh_d[:]).then_inc(s_out, 16)
                return
            for g in range(NG):
                nb_g = blocks_thru(g) - g * GRP
                rows = nb_g * P
                if g < 2:
                    sp.wait_ge(s_epi, K * NG)
                else:
                    sp.wait_ge(s_sm, g - 1)
                sp.dma_start(
                    zin_sb[g % 2][:, :nb_g * C].rearrange("p (j c) -> p j c", c=C),
                    zsh_d[g * NT:g * NT + rows, :].rearrange("(j p) c -> p j c", p=P),
                ).then_inc(s_zin, 16)
                sp.wait_ge(s_sm, g + 1)
                sp.dma_start(
                    out_d[g * NT:g * NT + rows, :].rearrange("(j p) c -> p j c", p=P),
                    sm_sb[g % 2][:, :nb_g * C].rearrange("p (j c) -> p j c", c=C),
                ).then_inc(s_out, 16)

        # ---------------- POOL ----------------
        @block.gpsimd
        def _(po):
            po.load_library(mlp_lib)
            po.wait_ge(s_znw, 16 * NG)
            if DBG_NO_AG:
                po.dma_start(zfull[0][:SHP, :], zsh_d[:]).then_inc(s_dbg, 16)
                po.wait_ge(s_dbg, 16)
                po.sem_inc(s_cc, 1)
            else:
                po.collective_compute(
                "AllGather", mybir.AluOpType.bypass,
                replica_groups=[list(range(NCORES))],
                    ins=[zsh_d[:].opt()], outs=[zfull[0][:].opt()],
                ).then_inc(s_cc, 1)
            for k in range(K):
                po.wait_ge(s_cc, k + 1)
                zf = zfull[k % 2]
                for ci, (q, nt, tcol, wb, g) in enumerate(calls):
                    gb = k * NG + g
                    if ci == first_call_of_grp[g]:
                        if gb >= 2:
                            po.wait_ge(s_scale, gb - 1)
                        po.wait_ge(s_idx, 32 * (gb + 1))
                    qs = k * CPQ[q] + qseq[ci]
                    if qs >= GDEPTH:
                        po.wait_ge(sg[q], GINC * (qs - GDEPTH + 1))
                    loc = tcol - int(gto[g])
                    if DBG_NO_GATHER:
                        po.sem_inc(sg[q], GINC)
                    else:
                        po.dma_gather(
                            gbuf[gb % 2][:, loc * C:(loc + nt) * C].rearrange(
                                "p (s c) -> p s c", c=C),
                            zf[wb:min(wb + WINROWS, NEFF), :],
                            idxb[gb % NIB][:, loc * 8:(loc + nt) * 8],
                            nt * P, nt * P, C, queue_num=q,
                            single_packet=SP_FLAG,
                        ).then_inc(sg[q], 16)
                po.wait_ge(s_znw, 16 * NG * (k + 2))
                for q in range(NQ):
                    po.wait_ge(sg[q], GINC * (k + 1) * CPQ[q])
                if DBG_NO_AG:
                    po.dma_start(zfull[(k + 1) % 2][:SHP, :], zsh_d[:]).then_inc(s_dbg, 16)
                    po.wait_ge(s_dbg, 16 * (k + 2))
                    po.sem_inc(s_cc, 1)
                else:
                    po.collective_compute(
                        "AllGather", mybir.AluOpType.bypass,
                        replica_groups=[list(range(NCORES))],
                        ins=[zsh_d[:].opt()], outs=[zfull[(k + 1) % 2][:].opt()],
                    ).then_inc(s_cc, 1)

        # ---------------- PE ----------------
        @block.tensor
        def _(pe):
            pe.wait_ge(s_boot, 16 * NBOOT)
            for t in range(ntile):
                w = tw(t)
                pe.wait_ge(s_x, 16 * (t + 1))
                if t >= 2:
                    pe.wait_ge(s_act1, 2 * t - 2)
                hb = 0 if t % 2 == 0 else 4
                for m in range(MH):
                    for cch in range(KC):
                        mm = pe.matmul(
                            psum[:, (hb + m) * 512:(hb + m) * 512 + w],
                            lhsT=w1_sb[:, cch, m * P:(m + 1) * P],
                            rhs=xt_sb[t % 2][:, cch, :w],
                            start=(cch == 0), stop=(cch == KC - 1),
                        )
                mm.then_inc(s_pe1, 1)
                pe.wait_ge(s_act1, 2 * t + 2)
                if t >= 2:
                    pe.wait_ge(s_fz, t - 1)
                    pe.wait_ge(s_act3, t - 1)
                zb = 2 if t % 2 == 0 else 6
                nj = (w + P - 1) // P
                for j in range(nj):
                    jw = min(P, w - j * P)
                    for m in range(MH):
                        mm = pe.matmul(
                            psum[:jw, zb * 512 + j * C:zb * 512 + j * C + C],
                            lhsT=ht_sb[t % 2][:, m, j * P:j * P + jw],
                            rhs=w2_sb[:, m, :],
                            start=(m == 0), stop=(m == MH - 1),
                        )
                mm.then_inc(s_pe2, 1)
            for k in range(K):
                for g in range(NG):
                    gb = k * NG + g
                    if gb < 2:
                        pe.wait_ge(s_fz, NG)
                        pe.wait_ge(s_act3, NG)
                    else:
                        pe.wait_ge(s_epi, gb - 1)
                    pe.wait_ge(s_scale, gb + 1)
                    for b in range(g * GRP, blocks_thru(g)):
                        bank = b % 8
                        ch = blk_chunks[b]
                        for ci2, (tcol, nt) in enumerate(ch):
                            loc = tcol - int(gto[g])
                            mm = pe.matmul(
                                psum[:, bank * 512:bank * 512 + nt * C],
                                lhsT=idbf[:],
                                rhs=mbuf[:, loc * C:(loc + nt) * C],
                                start=(ci2 == 0), stop=(ci2 == len(ch) - 1),
                                skip_group_check=True,
                            )
                        mm.then_inc(s_mm, 1)

        # ---------------- DVE ----------------
        @block.vector
        def _(ve):
            dvec = [0]

            def dtick():
                dvec[0] += 1
                return dvec[0]

            ve.wait_ge(s_boot, 16 * NBOOT)
            for t in range(ntile):
                w = tw(t)
                nj = (w + P - 1) // P
                ve.wait_ge(s_pe2, t + 1)
                if t >= 2:
                    ve.wait_ge(s_znw, 16 * (t - 1))
                    ve.wait_ge(s_act3, t - 1)
                zb = 2 if t % 2 == 0 else 6
                ve.tensor_tensor(
                    out=zf_sb[t % 2][:, :nj * C].rearrange("p (j c) -> p j c", c=C),
                    in0=psum[:, zb * 512:zb * 512 + nj * C].rearrange("p (j c) -> p j c", c=C),
                    in1=b2_sb[:].rearrange("p (a c) -> p a c", a=1).to_broadcast([P, nj, C]),
                    op=mybir.AluOpType.add,
                ).then_inc(s_fz, 1)
            for k in range(K):
                for g in range(NG):
                    gb = k * NG + g
                    tlo = int(gto[g])
                    tg = int(gto[g + 1] - gto[g])
                    # +1 call of slack per queue: the gather's completion sem
                    # can fire marginally before its last SBUF writes land;
                    # queue rings are in-order, so the NEXT call's completion
                    # implies this group's writes are visible.
                    for q in range(NQ):
                        ve.wait_ge(sg[q], GINC * min(
                            k * CPQ[q] + int(cumq[g][q]) + 2,
                            (k + 1) * CPQ[q]))
                    mmprev = k * NB + (blocks_thru(g - 1) if g >= 1 else 0)
                    if mmprev > 0:
                        ve.wait_ge(s_mm, mmprev)
                    ve.tensor_tensor(
                        out=mbuf[:, :tg * C].rearrange("p (s c) -> p s c", c=C),
                        in0=gbuf[gb % 2][:, :tg * C].rearrange("p (s c) -> p s c", c=C),
                        in1=nrmb[gb % NIB][:, :tg].to_broadcast([P, tg, C]),
                        op=mybir.AluOpType.mult,
                    ).then_inc(s_scale, 1)
                    nb_g = blocks_thru(g) - g * GRP
                    ve.wait_ge(s_zin, 32 * (gb + 1))
                    ve.tensor_tensor(
                        out=tmp_sb[:, :nb_g * C].rearrange("p (B c) -> p B c", c=C),
                        in0=zin_sb[gb % 2][:, :nb_g * C].rearrange("p (B c) -> p B c", c=C),
                        in1=sdg_sb[:, g * GRP:g * GRP + nb_g].to_broadcast([P, nb_g, C]),
                        op=mybir.AluOpType.mult,
                    )
                    ve.wait_ge(s_mm, k * NB + blocks_thru(g))
                    pbase = ((g * GRP) % 8) * 512
                    ve.tensor_reduce(
                        out=agg_sb[:, :nb_g * C].rearrange("p (B c) -> p B c", c=C),
                        in_=psum[:, pbase:pbase + nb_g * 512].rearrange(
                            "p (B j c) -> p B c j", j=8, c=C),
                        axis=mybir.AxisListType.X, op=mybir.AluOpType.add,
                    ).then_inc(s_dve, 1)
                    ve.wait_ge(s_dve, dtick())
                    if gb >= 2:
                        ve.wait_ge(s_znw, 16 * (NG + gb - 1))
                    ve.tensor_tensor(
                        out=tmp_sb[:, :nb_g * C],
                        in0=tmp_sb[:, :nb_g * C],
                        in1=agg_sb[:, :nb_g * C],
                        op=mybir.AluOpType.add,
                    ).then_inc(s_dve, 1)
                    ve.wait_ge(s_dve, dtick())
                    ve.scalar_tensor_tensor(
                        out=znew_sb[gb % 2][:, :nb_g * C],
                        in0=tmp_sb[:, :nb_g * C],
                        scalar=1.0 - ALPHA,
                        in1=z0in_sb[gb % 2][:, :nb_g * C],
                        op0=mybir.AluOpType.mult,
                        op1=mybir.AluOpType.add,
                    ).then_inc(s_epi, 1)
            if DBG_NO_SOFTMAX:
                return
            for g in range(NG):
                nb_g = blocks_thru(g) - g * GRP
                ve.wait_ge(s_zin, 32 * K * NG + 16 * (g + 1))
                ve.tensor_reduce(
                    out=nm_sb[:, :nb_g],
                    in_=zin_sb[g % 2][:, :nb_g * C].rearrange("p (B c) -> p B c", c=C),
                    axis=mybir.AxisListType.X, op=mybir.AluOpType.max,
                    negate=True,
                ).then_inc(s_sm1, 1)
                ve.wait_ge(s_sm2, 2 * (g + 1))
                if g >= 2:
                    ve.wait_ge(s_out, 16 * (g - 1))
                for j in range(nb_g):
                    mm = ve.scalar_tensor_tensor(
                        out=sm_sb[g % 2][:, j * C:(j + 1) * C],
                        in0=zin_sb[g % 2][:, j * C:(j + 1) * C],
                        scalar=nm_sb[:, j:j + 1],
                        in1=lse_sb[:, j:j + 1].to_broadcast([P, C]),
                        op0=mybir.AluOpType.add,
                        op1=mybir.AluOpType.subtract,
                    )
                mm.then_inc(s_sm, 1)

        # ---------------- ACT ----------------
        @block.scalar
        def _(ac):
            ac.wait_ge(s_boot, 16 * NBOOT)
            for t in range(ntile):
                w = tw(t)
                hb = 0 if t % 2 == 0 else 4
                zb = 2 if t % 2 == 0 else 6
                ac.wait_ge(s_pe1, t + 1)
                if t >= 2:
                    ac.wait_ge(s_pe2, t - 1)
                for m in range(MH):
                    ac.activation(
                        out=ht_sb[t % 2][:, m, :w],
                        in_=psum[:, (hb + m) * 512:(hb + m) * 512 + w],
                        func=mybir.ActivationFunctionType.Relu,
                        bias=b1_sb[:, m:m + 1],
                    ).then_inc(s_act1, 1)
                nj = (w + P - 1) // P
                ac.wait_ge(s_fz, t + 1)
                if t >= 2:
                    ac.wait_ge(s_z0w, 16 * (t - 1))
                ac.activation(
                    out=z0o_sb[t % 2][:, :nj * C],
                    in_=zf_sb[t % 2][:, :nj * C],
                    func=mybir.ActivationFunctionType.Copy,
                    scale=ALPHA,
                ).then_inc(s_act3, 1)
            for k in range(K):
                for g in range(NG):
                    gb = k * NG + g
                    tlo, thi = int(gto[g]), int(gto[g + 1])
                    if gb >= NIB - 1:
                        ac.wait_ge(s_scale, gb - (NIB - 1))
                    ac.dma_start(idxb[gb % NIB][:, :(thi - tlo) * 8],
                                 idx_d[:, tlo * 8:thi * 8]).then_inc(s_idx, 16)
                    ac.dma_start(nrmb[gb % NIB][:, :thi - tlo],
                                 nrm_d[:, tlo:thi]).then_inc(s_idx, 16)
            if DBG_NO_SOFTMAX:
                return
            for g in range(NG):
                nb_g = blocks_thru(g) - g * GRP
                ac.wait_ge(s_sm1, g + 1)
                for j in range(nb_g):
                    ins_e = ac.activation(
                        out=esc_sb[:, j * C:(j + 1) * C],
                        in_=zin_sb[g % 2][:, j * C:(j + 1) * C],
                        func=mybir.ActivationFunctionType.Exp,
                        bias=nm_sb[:, j:j + 1],
                        accum_out=se_sb[:, j:j + 1],
                    )
                ins_e.then_inc(s_sm2, 1)
                ac.wait_ge(s_sm2, 2 * g + 1)
                ac.activation(
                    out=lse_sb[:, :nb_g],
                    in_=se_sb[:, :nb_g],
                    func=mybir.ActivationFunctionType.Ln,
                ).then_inc(s_sm2, 1)

    nc.compile()
    return nc


# ---------------------------------------------------------------------------
# execution: jit once, keep inputs device-resident
# ---------------------------------------------------------------------------

def _make_runner(nc, in_maps, n_cores):
    """Mirror concourse.bass2jax.run_bass_via_pjrt, but build the jitted
    callable once and keep the (concatenated, sharded) inputs resident on
    device, so repeated executions measure device time rather than host->
    device transfer of ~200MB over the axon tunnel."""
    import jax
    from concourse import bass2jax

    bass2jax.install_neuronx_cc_hook()
    partition_name = (nc.partition_id_tensor.name
                      if nc.partition_id_tensor else None)
    in_names, out_names, out_avals, zero_outs = [], [], [], []
    for alloc in nc.m.functions[0].allocations:
        if not isinstance(alloc, mybir.MemoryLocationSet):
            continue
        name = alloc.memorylocations[0].name
        if alloc.kind == "ExternalInput":
            if name != partition_name:
                in_names.append(name)
        elif alloc.kind == "ExternalOutput":
            out_names.append(name)
            shape = tuple(alloc.tensor_shape)
            dtype = mybir.dt.np(alloc.dtype)
            out_avals.append(jax.core.ShapedArray(shape, dtype))
            zero_outs.append(np.zeros(shape, dtype))
    n_params = len(in_names)
    all_in = list(in_names) + list(out_names)
    if partition_name is not None:
        all_in.append(partition_name)

    def _body(*args):
        operands = list(args)
        if partition_name is not None:
            operands.append(bass2jax.partition_id_tensor())
        outs = bass2jax._bass_exec_p.bind(
            *operands,
            out_avals=tuple(out_avals),
            in_names=tuple(all_in),
            out_names=tuple(out_names),
            lowering_input_output_aliases=(),
            sim_require_finite=True,
            sim_require_nnan=True,
            nc=nc,
        )
        return tuple(outs)

    devices = jax.devices()[:n_cores]
    mesh = bass2jax.Mesh(np.asarray(devices), ("core",))
    spec = bass2jax.PartitionSpec("core")
    concat_in = [
        np.concatenate([np.asarray(in_maps[c][nm]) for c in range(n_cores)],
                       axis=0)
        for nm in in_names]
    concat_zero = [np.zeros((n_cores * z.shape[0], *z.shape[1:]), z.dtype)
                   for z in zero_outs]
    sharding = jax.sharding.NamedSharding(mesh, spec)
    dev_in = [jax.device_put(a, sharding) for a in concat_in + concat_zero]
    jax.block_until_ready(dev_in)

    def _compile():
        return jax.jit(
            bass2jax.shard_map(
                _body, mesh=mesh,
                in_specs=(spec,) * (n_params + len(out_names)),
                out_specs=(spec,) * len(out_names),
                check_rep=False),
            keep_unused=True).lower(*dev_in).compile()

    jitted = bass2jax.fast_dispatch_compile(_compile)

    def dispatch():
        return jitted(*dev_in)

    def run():
        outs = jitted(*dev_in)
        jax.block_until_ready(outs)
        return outs
    run.dispatch = dispatch

    def fetch(outs):
        return [
            {nm: np.asarray(outs[i]).reshape(n_cores, *out_avals[i].shape)[c]
             for i, nm in enumerate(out_names)}
            for c in range(n_cores)]
    return run, fetch


# ---------------------------------------------------------------------------
# top level
# ---------------------------------------------------------------------------

def _make_in_maps(meta, inputs):
    D, C = meta["D"], meta["C"]
    MH = D // P
    b1p = np.zeros((P, MH), np.float32)
    b1 = np.asarray(inputs["b1"], np.float32)
    for m in range(MH):
        b1p[:, m] = b1[m * P:(m + 1) * P]
    b2b = np.tile(np.asarray(inputs["b2"], np.float32).reshape(1, C), (P, 1))
    idf32 = np.eye(P, dtype=np.float32)
    idbf = np.eye(P, dtype=np.float32).astype(ml_dtypes.bfloat16)

    in_maps = []
    for c in range(NCORES):
        cd = meta["core_data"][c]
        in_maps.append({
            "xT": np.asarray(cd["xT"]),
            "W1p": np.asarray(meta["W1p"]),
            "W2": np.asarray(meta["W2"]),
            "b1p": b1p, "b2b": b2b,
            "idf32": idf32, "idbf": np.asarray(idbf),
            "sdg": cd["s_sb"],
            "idxs": cd["idx_all"],
            "nrms": cd["norm_all"],
        })
    return in_maps


def _run(inputs, kiter):
    meta = _preprocess(
        inputs["x"], inputs["edge_index"], inputs["edge_weight_train"],
        inputs["x_weight"], inputs["W1"], inputs["b1"], inputs["W2"],
        inputs["b2"], kiter,
    )
    nc = _build(meta)
    in_maps = _make_in_maps(meta, inputs)

    meta["_nc"] = nc
    meta["_in_maps"] = in_maps
    run, fetch = _make_runner(nc, in_maps, NCORES)
    meta["_run_fn"] = run
    meta["_fetch_fn"] = fetch
    res_outs = run()
    results = fetch(res_outs)

    N, C = meta["N"], meta["C"]
    SHP = meta["SHP"]
    perm = meta["perm"]
    out_full = np.empty((N, C), np.float32)
    for c in range(NCORES):
        o = results[c]["out"]
        rows = np.arange(SHP)
        nodes = perm[c * SHP + rows]
        v = nodes >= 0
        out_full[nodes[v]] = o[rows[v]]
    return out_full, meta, results


def kernel(**inputs):
    out, _, _ = _run(inputs, KITER_DEFAULT)
    return out



# revision 44
# speedup vs baseline: 1.0469x; 1.0469x over previous
"""APPNP GNN kernel for 8 TRN2 NeuronCores — raw Bass engine programs.

Per core (dest-stationary sharding):
  GEMM:  h = relu(x@W1'), z0 = h@W2 + b2  (data-parallel over its nodes)
  K x propagation:
     dma_gather z rows (f32 256B rows, int16 idx in overlapping 32K windows)
     DVE: scale by edge norm -> bf16 messages
     PE:  identity-matmul accumulate into PSUM per 128-dest block
     DVE: z_new = 0.9*(agg + selfloop*z) + 0.1*z0
     AllGather bf16 shards -> replicated z_full
  log_softmax epilogue; host unpermutes rows.
"""

import numpy as np
import ml_dtypes

import concourse.bass as bass
import concourse.bacc as bacc
import concourse.mybir as mybir
from concourse.library_config import mlp as mlp_lib

F32 = mybir.dt.float32
BF16 = mybir.dt.bfloat16
I16 = mybir.dt.int16

NCORES = 8
P = 128
ALPHA = 0.1

CALL_TILES = 8     # tiles per dma_gather call (1024 idxs; safe at depth 2)
GDEPTH = 4
SP_FLAG = True
NQ = 4
GRP = 2            # dest blocks per group
WINROWS = 32768
KITER_DEFAULT = 10
GINC = 16   # sem increment per dma_gather on HW (sim models 32)
NIB_DOC = """idx/norm stream is prefetched 4-deep by the ACT engine; a dummy
flush gather per queue closes each iteration so completion sems can't lead
the data writes observed by DVE."""
DBG_NO_SOFTMAX = False
DBG_NO_AG = False
DBG_NO_GATHER = False
DBG_NO_LEVELOPT = False


def _sigmoid(v):
    return 1.0 / (1.0 + np.exp(-v.astype(np.float64)))


# ---------------------------------------------------------------------------
# host preprocessing
# ---------------------------------------------------------------------------

def _preprocess(x, edge_index, edge_weight_train, x_weight, W1, b1, W2, b2, kiter):
    N, F = x.shape
    D = W1.shape[1]
    C = W2.shape[1]
    row = np.asarray(edge_index[0], np.int64)
    col = np.asarray(edge_index[1], np.int64)

    ew = np.asarray(edge_weight_train, np.float32)
    mask = np.abs(ew) > 0
    w = np.where(mask, _sigmoid(ew), 0.0).astype(np.float32)
    deg = (np.bincount(col, weights=w.astype(np.float64), minlength=N) + 1.0).astype(np.float32)
    dinv = np.where(deg > 0, 1.0 / np.sqrt(np.maximum(deg, 1e-12)), 0.0).astype(np.float32)
    enorm = (dinv[row] * w * dinv[col]).astype(np.float32)
    sdiag = (dinv * dinv).astype(np.float32)

    xw = (_sigmoid(np.asarray(x_weight, np.float32))
          * (np.abs(x_weight) > 0)).astype(np.float32)
    W1p = xw[:, None] * np.asarray(W1, np.float32)

    assert N % NCORES == 0
    SH = N // NCORES
    NB = (SH + P - 1) // P
    SHP = NB * P
    NEFF = SHP * NCORES
    cnt = np.bincount(col, minlength=N)
    order = np.argsort(-cnt, kind="stable")
    pat = np.concatenate([np.arange(NCORES), np.arange(NCORES)[::-1]])
    core_of_rank = pat[np.arange(N) % (2 * NCORES)]
    if np.bincount(core_of_rank, minlength=NCORES).max() != SH:
        core_of_rank = np.repeat(np.arange(NCORES), SH)
    pos = np.empty(N, np.int64)
    core_of_node = np.empty(N, np.int64)
    for c in range(NCORES):
        nodes_c = order[core_of_rank == c]
        pos[nodes_c] = c * SHP + np.arange(len(nodes_c))
        core_of_node[nodes_c] = c

    if NEFF <= WINROWS:
        bases = np.array([0], np.int64)
    else:
        nw = 4
        bases = np.array([int(round(k * (NEFF - WINROWS) / (nw - 1)))
                          for k in range(nw)], np.int64)
    NW = len(bases)
    ND = NEFF

    def _window_fill(pos_a):
        src_pos = pos_a[row]
        dloc = pos_a[col]
        wlo = np.searchsorted(bases + WINROWS - 1, src_pos, side="left")
        whi = np.searchsorted(bases, src_pos, side="right") - 1
        assert (wlo <= whi).all()
        f = np.zeros((NW, ND), np.int64)
        flex = np.zeros((max(NW - 1, 1), ND), np.int64)
        forced_m = wlo == whi
        for wi in range(NW):
            np.add.at(f[wi], dloc[forced_m & (wlo == wi)], 1)
        for wi in range(NW - 1):
            np.add.at(flex[wi], dloc[(~forced_m) & (wlo == wi)], 1)
        degp = np.bincount(dloc, minlength=ND)
        tgt = (degp + NW - 1) // NW
        takes = np.zeros((max(NW - 1, 1), ND), np.int64)
        cw = np.zeros((NW, ND), np.int64)
        carry = np.zeros(ND, np.int64)
        for wi in range(NW):
            base_c = f[wi] + carry
            if wi < NW - 1:
                take = np.clip(tgt - base_c, 0, flex[wi])
                takes[wi] = take
                carry = flex[wi] - take
                cw[wi] = base_c + take
            else:
                cw[wi] = base_c
        return src_pos, dloc, wlo, whi, forced_m, f, flex, takes, cw, degp

    # profile-clustered block assignment: 2 refinement passes grouping dests
    # with similar (deg, per-window count) profiles into the same block to
    # shrink max-over-partitions padding (~15% less gather traffic).
    if NW > 1:
        for _it in range(2):
            _, _, _, _, _, _, _, _, cw_i, _ = _window_fill(pos)
            for c in range(NCORES):
                nodes_c = np.nonzero(core_of_node == c)[0]
                key = (cnt[nodes_c] * 32768
                       + cw_i[0][pos[nodes_c]] * 1024
                       + cw_i[1][pos[nodes_c]] * 32
                       + cw_i[2][pos[nodes_c]])
                o = np.argsort(-key, kind="stable")
                pos[nodes_c[o]] = c * SHP + np.arange(len(nodes_c))

    perm = np.full(NEFF, -1, np.int64)
    perm[pos] = np.arange(N)
    src_pos, dloc, wlo, whi, forced_m, f, flex, takes, cw, degp = _window_fill(pos)
    dst_pos = dloc

    # Per-block level optimization of the flex window assignment: for each
    # dest block, greedily lower the per-window level vector L (shared across
    # cores) subject to per-node chain feasibility, then re-derive takes/cw.
    # Cuts ~4% of gather padding vs the per-node tgt waterfill.
    if NW > 1 and not DBG_NO_LEVELOPT:
        b_of_all = (np.arange(NEFF) % SHP) // P

        def _chain(fb, flb, L):
            M = fb.shape[1]
            carry = np.zeros(M, np.int64)
            tk = np.zeros((NW - 1, M), np.int64)
            for w in range(NW):
                mustv = fb[w] + carry
                if (mustv > L[w]).any():
                    return None
                if w < NW - 1:
                    t = np.minimum(flb[w], L[w] - mustv)
                    tk[w] = t
                    carry = flb[w] - t
            return tk

        new_takes = np.zeros_like(takes)
        for b in range(NB):
            sel = b_of_all == b
            fb = f[:, sel]
            flb = flex[:, sel]
            cands = []
            for Linit in (cw[:, sel].max(axis=1),):
                L = Linit.astype(np.int64).copy()
                assert _chain(fb, flb, L) is not None
                improved = True
                while improved:
                    improved = False
                    for w in np.argsort(-L):
                        if L[w] == 0:
                            continue
                        L[w] -= 1
                        if _chain(fb, flb, L) is None:
                            L[w] += 1
                        else:
                            improved = True
                cands.append((int(L.sum()), L))
            L = min(cands)[1]
            new_takes[:, sel] = _chain(fb, flb, L)
        takes = new_takes
        carry = np.zeros(NEFF, np.int64)
        for wi in range(NW):
            base_c = f[wi] + carry
            if wi < NW - 1:
                carry = flex[wi] - takes[wi]
                cw[wi] = base_c + takes[wi]
            else:
                cw[wi] = base_c

    cw3 = cw.reshape(NW, NCORES, NB, P)
    T = cw3.max(axis=(1, 3)).T          # [NB, NW] shared tile table
    # every block must own one full 8-tile chunk (start=True zeroes the bank)
    for b in range(NB):
        wmax = int(np.argmax(T[b]))
        if T[b, wmax] < 8:
            T[b, wmax] = 8

    ngroups = (NB + GRP - 1) // GRP
    tile_off = np.zeros((NB, NW), np.int64)
    grp_tile_off = np.zeros(ngroups + 1, np.int64)
    tcol = 0
    for g in range(ngroups):
        grp_tile_off[g] = tcol
        for wi in range(NW):
            for b in range(g * GRP, min((g + 1) * GRP, NB)):
                tile_off[b, wi] = tcol
                tcol += T[b, wi]
    grp_tile_off[ngroups] = tcol
    T_tot = int(tcol)

    calls = []
    qn = 0
    grp_call_end = np.zeros(ngroups, np.int64)
    cumq = np.zeros((ngroups, NQ), np.int64)
    qcount = np.zeros(NQ, np.int64)
    for g in range(ngroups):
        for wi in range(NW):
            lo = None
            n = 0
            for b in range(g * GRP, min((g + 1) * GRP, NB)):
                if T[b, wi]:
                    if lo is None:
                        lo = tile_off[b, wi]
                    n += T[b, wi]
            t = 0
            while t < n:
                nt = min(CALL_TILES, n - t)
                calls.append((qn % NQ, int(nt), int(lo + t), int(bases[wi]), g))
                qcount[qn % NQ] += 1
                qn += 1
                t += nt
        grp_call_end[g] = len(calls)
        cumq[g] = qcount

    blk_chunks = []
    for b in range(NB):
        ch = []
        worder = np.argsort(-T[b], kind="stable")
        for wi in worder:
            t = 0
            while t < T[b, wi]:
                nt = min(8, T[b, wi] - t)
                ch.append((int(tile_off[b, wi] + t), int(nt)))
                t += nt
        assert not ch or ch[0][1] == 8
        blk_chunks.append(ch)

    # edge -> (window, rank-in-window) consistent with the waterfill
    win = np.where(forced_m, wlo, 0)
    flex_m = ~forced_m
    fkey = np.where(flex_m, wlo, -1)
    okey = dloc * (2 * NW) + np.where(flex_m, NW + fkey, win)
    eord = np.argsort(okey, kind="stable")
    sk = okey[eord]
    grp_start = np.r_[0, np.nonzero(np.diff(sk))[0] + 1]
    starts_full = np.zeros(len(sk), np.int64)
    starts_full[grp_start] = grp_start
    np.maximum.accumulate(starts_full, out=starts_full)
    rank_sorted = np.arange(len(sk)) - starts_full
    rank = np.empty(len(sk), np.int64)
    rank[eord] = rank_sorted

    ewin = np.where(forced_m, wlo, -1)
    if NW > 1:
        tk = takes[np.clip(fkey, 0, NW - 2), dloc]
        ewin = np.where(flex_m, np.where(rank < tk, wlo, wlo + 1), ewin)
    r_in_w = rank.copy()
    if NW > 1:
        carry_prev = np.zeros((NW, ND), np.int64)
        for wi in range(1, NW):
            carry_prev[wi] = flex[wi - 1] - takes[wi - 1]
        up = flex_m & (ewin == fkey)
        dn = flex_m & (ewin == fkey + 1)
        r_in_w[up] = f[ewin[up], dloc[up]] + carry_prev[ewin[up], dloc[up]] + rank[up]
        r_in_w[dn] = f[ewin[dn], dloc[dn]] + (rank[dn] - takes[np.clip(fkey[dn], 0, NW - 2), dloc[dn]])
    assert (r_in_w < cw[ewin, dloc]).all()

    b_of = (dloc % SHP) // P
    p_of = dloc % P
    core_of = dloc // SHP
    tcol_of = tile_off[b_of, ewin] + r_in_w
    idx16 = (src_pos - bases[ewin]).astype(np.int64)
    assert (idx16 >= 0).all() and (idx16 < WINROWS).all()

    core_data = []
    xf = np.asarray(x, np.float32)
    for c in range(NCORES):
        sel = core_of == c
        ig = np.zeros((T_tot, P), np.int16)
        ng = np.zeros((P, T_tot), np.float32)
        ig[tcol_of[sel], p_of[sel]] = idx16[sel].astype(np.int16)
        ng[p_of[sel], tcol_of[sel]] = enorm[sel]
        idx_cols = np.ascontiguousarray(ig.reshape(-1).reshape(-1, 16).T)
        idx_all = np.ascontiguousarray(np.tile(idx_cols, (8, 1)))
        s_sb = np.zeros((P, NB), np.float32)
        rows = np.arange(SHP)
        nodes = perm[c * SHP + rows]
        valid = nodes >= 0
        s_sb[rows[valid] % P, rows[valid] // P] = sdiag[nodes[valid]]
        xsh = np.zeros((SHP, F), np.float32)
        xsh[rows[valid]] = xf[nodes[valid]]
        xT = np.ascontiguousarray(xsh.T).astype(ml_dtypes.bfloat16)
        core_data.append(dict(idx_all=idx_all, norm_all=np.ascontiguousarray(ng),
                              s_sb=s_sb, xT=xT))

    return dict(
        N=N, F=F, D=D, C=C, SH=SH, SHP=SHP, NB=NB, NEFF=NEFF, NW=NW,
        ngroups=ngroups, T_tot=T_tot, kiter=kiter,
        bases=bases, calls=calls, cumq=cumq, grp_call_end=grp_call_end,
        grp_tile_off=grp_tile_off, blk_chunks=blk_chunks, T=T,
        perm=perm, pos=pos, core_data=core_data,
        W1p=W1p.astype(ml_dtypes.bfloat16),
        W2=np.asarray(W2, np.float32).astype(ml_dtypes.bfloat16),
    )


# ---------------------------------------------------------------------------
# device program
# ---------------------------------------------------------------------------

def _build(meta):
    F, D, C, SHP, NB, NEFF = (meta[k] for k in ("F", "D", "C", "SHP", "NB", "NEFF"))
    K = meta["kiter"]
    NG = meta["ngroups"]
    calls = meta["calls"]
    cumq = meta["cumq"]
    gto = meta["grp_tile_off"]
    blk_chunks = meta["blk_chunks"]
    T_tot = meta["T_tot"]
    KC = F // P
    MH = D // P
    NT = GRP * P
    ntile = NG
    Tg_max = int(max(gto[g + 1] - gto[g] for g in range(NG)))
    qseq = []
    qc = [0] * NQ
    first_call_of_grp = {}
    for ci, (q, nt, tcol, wb, g) in enumerate(calls):
        if g not in first_call_of_grp:
            first_call_of_grp[g] = ci
        qseq.append(qc[q])
        qc[q] += 1
    # one dummy flush gather per queue closes each iteration: its completion
    # sem (in-order ring) guarantees every real call's SBUF writes landed.
    CPQ = [c + 1 for c in qc]
    last_call = calls[-1]

    nc = bacc.Bacc("TRN2", target_bir_lowering=False, debug=False,
                   num_devices=NCORES, num_swdge_queues=NQ)

    xT_d = nc.dram_tensor("xT", [F, SHP], BF16, kind="ExternalInput")
    w1_d = nc.dram_tensor("W1p", [F, D], BF16, kind="ExternalInput")
    w2_d = nc.dram_tensor("W2", [D, C], BF16, kind="ExternalInput")
    b1_d = nc.dram_tensor("b1p", [P, MH], F32, kind="ExternalInput")
    b2_d = nc.dram_tensor("b2b", [P, C], F32, kind="ExternalInput")
    idf32_d = nc.dram_tensor("idf32", [P, P], F32, kind="ExternalInput")
    idbf_d = nc.dram_tensor("idbf", [P, P], BF16, kind="ExternalInput")
    sdg_d = nc.dram_tensor("sdg", [P, NB], F32, kind="ExternalInput")
    idx_d = nc.dram_tensor("idxs", [P, T_tot * 8], I16, kind="ExternalInput")
    nrm_d = nc.dram_tensor("nrms", [P, T_tot], F32, kind="ExternalInput")
    out_d = nc.dram_tensor("out", [SHP, C], F32, kind="ExternalOutput")

    zfull = [nc.dram_tensor(f"zfull{i}", [NEFF, C], F32, addr_space="Shared")
             for i in range(2)]
    zsh_d = nc.dram_tensor("zsh", [SHP, C], F32)
    z0s_d = nc.dram_tensor("z0s", [SHP, C], F32)

    A = nc.alloc_sbuf_tensor
    w1_sb = A("w1_sb", [P, KC, D], BF16)
    w2_sb = A("w2_sb", [P, MH, C], BF16)
    b1_sb = A("b1_sb", [P, MH], F32)
    b2_sb = A("b2_sb", [P, C], F32)
    idf32 = A("idf32_sb", [P, P], F32)
    idbf = A("idbf_sb", [P, P], BF16)
    sdg_sb = A("sdg_sb", [P, NB], F32)
    xt_sb = [A(f"xt{i}", [P, KC, NT], BF16) for i in range(2)]
    ht_sb = [A(f"ht{i}", [P, MH, NT], BF16) for i in range(2)]
    zf_sb = [A(f"zf{i}", [P, GRP * C], F32) for i in range(2)]
    z0o_sb = [A(f"z0o{i}", [P, GRP * C], F32) for i in range(2)]
    NGB = 3
    gbuf = [A(f"gbuf{i}", [P, Tg_max * C], F32) for i in range(NGB)]
    mbuf = A("mbuf", [P, Tg_max * C], BF16)
    # idx/norm tables resident in SBUF: loaded once at boot, reused all K
    # iterations (saves ~12MB/iter of HBM re-DMA + per-group sync).
    # 4-deep idx/norm stream buffers, prefetched by the ACT engine so the
    # gather stream never queues behind SP's z-traffic waits.
    NIB = 4
    idxb = [A(f"idxb{i}", [P, Tg_max * 8], I16) for i in range(NIB)]
    nrmb = [A(f"nrmb{i}", [P, Tg_max], F32) for i in range(NIB)]
    scr_sb = A("scr_sb", [P, C], F32)
    zin_sb = [A(f"zin{i}", [P, GRP * C], F32) for i in range(2)]
    z0in_sb = [A(f"z0in{i}", [P, GRP * C], F32) for i in range(2)]
    znew_sb = [A(f"znew{i}", [P, GRP * C], F32) for i in range(2)]
    agg_sb = A("agg_sb", [P, GRP * C], F32)
    tmp_sb = A("tmp_sb", [P, GRP * C], F32)
    nm_sb = A("nm_sb", [P, GRP], F32)
    se_sb = A("se_sb", [P, GRP], F32)
    lse_sb = A("lse_sb", [P, GRP], F32)
    esc_sb = A("esc_sb", [P, GRP * C], F32)
    sm_sb = [A(f"sm{i}", [P, GRP * C], F32) for i in range(2)]

    psum = nc.alloc_psum_tensor("psumall", [P, 4096], F32)

    S = nc.alloc_semaphore
    s_boot = S("s_boot"); s_idx = S("s_idx"); s_scale = S("s_scale")
    s_mm = S("s_mm"); s_epi = S("s_epi"); s_ms = S("s_ms")
    s_zin = S("s_zin"); s_znw = S("s_znw"); s_z0w = S("s_z0w")
    s_cc = S("s_cc"); s_x = S("s_x")
    s_pe1 = S("s_pe1"); s_pe2 = S("s_pe2"); s_pe3 = S("s_pe3")
    s_act1 = S("s_act1"); s_act2 = S("s_act2"); s_act3 = S("s_act3")
    s_fz = S("s_fz"); s_sm1 = S("s_sm1"); s_sm2 = S("s_sm2")
    s_sm = S("s_sm"); s_out = S("s_out"); s_dve = S("s_dve"); s_dbg = S("s_dbg")
    sg = [S(f"sg{q}") for q in range(NQ)]

    NBOOT = 7

    def blocks_thru(g):
        return min((g + 1) * GRP, NB)

    def tw(t):
        return min(NT, SHP - t * NT)

    with nc.Block() as block:

        # ---------------- SP ----------------
        @block.sync
        def _(sp):
            sp.dma_start(w1_sb[:], w1_d[:].rearrange("(kc p) m -> p kc m", p=P)).then_inc(s_boot, 16)
            sp.dma_start(w2_sb[:], w2_d[:].rearrange("(mh p) m -> p mh m", p=P)).then_inc(s_boot, 16)
            sp.dma_start(b1_sb[:], b1_d[:]).then_inc(s_boot, 16)
            sp.dma_start(b2_sb[:], b2_d[:]).then_inc(s_boot, 16)
            sp.dma_start(idf32[:], idf32_d[:]).then_inc(s_boot, 16)
            sp.dma_start(idbf[:], idbf_d[:]).then_inc(s_boot, 16)
            sp.dma_start(sdg_sb[:], sdg_d[:]).then_inc(s_boot, 16)

            for t in range(ntile + 2):
                if t < ntile:
                    w = tw(t)
                    if t >= 2:
                        sp.wait_ge(s_pe1, t - 1)
                    sp.dma_start(
                        xt_sb[t % 2][:, :, :w],
                        xT_d[:].rearrange("(kc p) n -> p kc n", p=P)[:, :, t * NT:t * NT + w],
                    ).then_inc(s_x, 16)
                if t >= 2:
                    u = t - 2
                    w = tw(u)
                    sp.wait_ge(s_fz, u + 1)
                    sp.dma_start(
                        zsh_d[u * NT:u * NT + w, :].rearrange("(j p) c -> p j c", p=P),
                        zf_sb[u % 2][:, :(w // P) * C].rearrange("p (j c) -> p j c", c=C),
                    ).then_inc(s_znw, 16)
                    sp.wait_ge(s_act3, u + 1)
                    sp.dma_start(
                        z0s_d[u * NT:u * NT + w, :].rearrange("(j p) c -> p j c", p=P),
                        z0o_sb[u % 2][:, :(w // P) * C].rearrange("p (j c) -> p j c", c=C),
                    ).then_inc(s_z0w, 16)

            for k in range(K):
                for g in range(NG):
                    gb = k * NG + g
                    nb_g = blocks_thru(g) - g * GRP
                    rows = nb_g * P
                    if gb >= 2:
                        sp.wait_ge(s_epi, gb - 1)
                    sp.wait_ge(s_znw, 16 * (k * NG + g + 1))
                    sp.dma_start(
                        zin_sb[gb % 2][:, :nb_g * C].rearrange("p (j c) -> p j c", c=C),
                        zsh_d[g * NT:g * NT + rows, :].rearrange("(j p) c -> p j c", p=P),
                    ).then_inc(s_zin, 16)
                    sp.wait_ge(s_z0w, 16 * min(g + 1, NG))
                    sp.dma_start(
                        z0in_sb[gb % 2][:, :nb_g * C].rearrange("p (j c) -> p j c", c=C),
                        z0s_d[g * NT:g * NT + rows, :].rearrange("(j p) c -> p j c", p=P),
                    ).then_inc(s_zin, 16)
                    sp.wait_ge(s_epi, gb + 1)
                    sp.dma_start(
                        zsh_d[g * NT:g * NT + rows, :].rearrange("(j p) c -> p j c", p=P),
                        znew_sb[gb % 2][:, :nb_g * C].rearrange("p (j c) -> p j c", c=C),
                    ).then_inc(s_znw, 16)

            if DBG_NO_SOFTMAX:
                sp.wait_ge(s_epi, K * NG)
                sp.wait_ge(s_znw, 16 * (K + 1) * NG)
                sp.dma_start(out_d[:], zsh_d[:]).then_inc(s_out, 16)
                return
            for g in range(NG):
                nb_g = blocks_thru(g) - g * GRP
                rows = nb_g * P
                if g < 2:
                    sp.wait_ge(s_epi, K * NG)
                else:
                    sp.wait_ge(s_sm, g - 1)
                sp.dma_start(
                    zin_sb[g % 2][:, :nb_g * C].rearrange("p (j c) -> p j c", c=C),
                    zsh_d[g * NT:g * NT + rows, :].rearrange("(j p) c -> p j c", p=P),
                ).then_inc(s_zin, 16)
                sp.wait_ge(s_sm, g + 1)
                sp.dma_start(
                    out_d[g * NT:g * NT + rows, :].rearrange("(j p) c -> p j c", p=P),
                    sm_sb[g % 2][:, :nb_g * C].rearrange("p (j c) -> p j c", c=C),
                ).then_inc(s_out, 16)

        # ---------------- POOL ----------------
        @block.gpsimd
        def _(po):
            po.load_library(mlp_lib)
            po.wait_ge(s_znw, 16 * NG)
            if DBG_NO_AG:
                po.dma_start(zfull[0][:SHP, :], zsh_d[:]).then_inc(s_dbg, 16)
                po.wait_ge(s_dbg, 16)
                po.sem_inc(s_cc, 1)
            else:
                po.collective_compute(
                "AllGather", mybir.AluOpType.bypass,
                replica_groups=[list(range(NCORES))],
                    ins=[zsh_d[:].opt()], outs=[zfull[0][:].opt()],
                ).then_inc(s_cc, 1)
            for k in range(K):
                po.wait_ge(s_cc, k + 1)
                zf = zfull[k % 2]
                for ci, (q, nt, tcol, wb, g) in enumerate(calls):
                    gb = k * NG + g
                    if ci == first_call_of_grp[g]:
                        if gb >= NGB:
                            po.wait_ge(s_scale, gb - (NGB - 1))
                        po.wait_ge(s_idx, 32 * (gb + 1))
                    qs = k * CPQ[q] + qseq[ci]
                    if qs >= GDEPTH:
                        po.wait_ge(sg[q], GINC * (qs - GDEPTH + 1))
                    loc = tcol - int(gto[g])
                    if DBG_NO_GATHER:
                        po.sem_inc(sg[q], GINC)
                    else:
                        po.dma_gather(
                            gbuf[gb % NGB][:, loc * C:(loc + nt) * C].rearrange(
                                "p (s c) -> p s c", c=C),
                            zf[wb:min(wb + WINROWS, NEFF), :],
                            idxb[gb % NIB][:, loc * 8:(loc + nt) * 8],
                            nt * P, nt * P, C, queue_num=q,
                            single_packet=SP_FLAG,
                        ).then_inc(sg[q], 16)
                po.wait_ge(s_znw, 16 * NG * (k + 2))
                for q in range(NQ):
                    po.wait_ge(sg[q], GINC * (k + 1) * CPQ[q])
                if DBG_NO_AG:
                    po.dma_start(zfull[(k + 1) % 2][:SHP, :], zsh_d[:]).then_inc(s_dbg, 16)
                    po.wait_ge(s_dbg, 16 * (k + 2))
                    po.sem_inc(s_cc, 1)
                else:
                    po.collective_compute(
                        "AllGather", mybir.AluOpType.bypass,
                        replica_groups=[list(range(NCORES))],
                        ins=[zsh_d[:].opt()], outs=[zfull[(k + 1) % 2][:].opt()],
                    ).then_inc(s_cc, 1)

        # ---------------- PE ----------------
        @block.tensor
        def _(pe):
            pe.wait_ge(s_boot, 16 * NBOOT)
            for t in range(ntile):
                w = tw(t)
                pe.wait_ge(s_x, 16 * (t + 1))
                if t >= 2:
                    pe.wait_ge(s_act1, 2 * t - 2)
                hb = 0 if t % 2 == 0 else 4
                for m in range(MH):
                    for cch in range(KC):
                        mm = pe.matmul(
                            psum[:, (hb + m) * 512:(hb + m) * 512 + w],
                            lhsT=w1_sb[:, cch, m * P:(m + 1) * P],
                            rhs=xt_sb[t % 2][:, cch, :w],
                            start=(cch == 0), stop=(cch == KC - 1),
                        )
                mm.then_inc(s_pe1, 1)
                pe.wait_ge(s_act1, 2 * t + 2)
                if t >= 2:
                    pe.wait_ge(s_fz, t - 1)
                    pe.wait_ge(s_act3, t - 1)
                zb = 2 if t % 2 == 0 else 6
                nj = (w + P - 1) // P
                for j in range(nj):
                    jw = min(P, w - j * P)
                    for m in range(MH):
                        mm = pe.matmul(
                            psum[:jw, zb * 512 + j * C:zb * 512 + j * C + C],
                            lhsT=ht_sb[t % 2][:, m, j * P:j * P + jw],
                            rhs=w2_sb[:, m, :],
                            start=(m == 0), stop=(m == MH - 1),
                        )
                mm.then_inc(s_pe2, 1)
            for k in range(K):
                for g in range(NG):
                    gb = k * NG + g
                    if gb < 2:
                        pe.wait_ge(s_fz, NG)
                        pe.wait_ge(s_act3, NG)
                    else:
                        pe.wait_ge(s_epi, gb - 1)
                    pe.wait_ge(s_scale, gb + 1)
                    for b in range(g * GRP, blocks_thru(g)):
                        bank = b % 8
                        ch = blk_chunks[b]
                        for ci2, (tcol, nt) in enumerate(ch):
                            loc = tcol - int(gto[g])
                            mm = pe.matmul(
                                psum[:, bank * 512:bank * 512 + nt * C],
                                lhsT=idbf[:],
                                rhs=mbuf[:, loc * C:(loc + nt) * C],
                                start=(ci2 == 0), stop=(ci2 == len(ch) - 1),
                                skip_group_check=True,
                            )
                        mm.then_inc(s_mm, 1)

        # ---------------- DVE ----------------
        @block.vector
        def _(ve):
            dvec = [0]

            def dtick():
                dvec[0] += 1
                return dvec[0]

            ve.wait_ge(s_boot, 16 * NBOOT)
            for t in range(ntile):
                w = tw(t)
                nj = (w + P - 1) // P
                ve.wait_ge(s_pe2, t + 1)
                if t >= 2:
                    ve.wait_ge(s_znw, 16 * (t - 1))
                    ve.wait_ge(s_act3, t - 1)
                zb = 2 if t % 2 == 0 else 6
                ve.tensor_tensor(
                    out=zf_sb[t % 2][:, :nj * C].rearrange("p (j c) -> p j c", c=C),
                    in0=psum[:, zb * 512:zb * 512 + nj * C].rearrange("p (j c) -> p j c", c=C),
                    in1=b2_sb[:].rearrange("p (a c) -> p a c", a=1).to_broadcast([P, nj, C]),
                    op=mybir.AluOpType.add,
                ).then_inc(s_fz, 1)
            for k in range(K):
                for g in range(NG):
                    gb = k * NG + g
                    tlo = int(gto[g])
                    tg = int(gto[g + 1] - gto[g])
                    # +1 call of slack per queue: the gather's completion sem
                    # can fire marginally before its last SBUF writes land;
                    # queue rings are in-order, so the NEXT call's completion
                    # implies this group's writes are visible.
                    for q in range(NQ):
                        ve.wait_ge(sg[q], GINC * min(
                            k * CPQ[q] + int(cumq[g][q]) + 2,
                            (k + 1) * CPQ[q]))
                    mmprev = k * NB + (blocks_thru(g - 1) if g >= 1 else 0)
                    if mmprev > 0:
                        ve.wait_ge(s_mm, mmprev)
                    ve.tensor_tensor(
                        out=mbuf[:, :tg * C].rearrange("p (s c) -> p s c", c=C),
                        in0=gbuf[gb % NGB][:, :tg * C].rearrange("p (s c) -> p s c", c=C),
                        in1=nrmb[gb % NIB][:, :tg].to_broadcast([P, tg, C]),
                        op=mybir.AluOpType.mult,
                    ).then_inc(s_scale, 1)
                    nb_g = blocks_thru(g) - g * GRP
                    ve.wait_ge(s_zin, 32 * (gb + 1))
                    ve.tensor_tensor(
                        out=tmp_sb[:, :nb_g * C].rearrange("p (B c) -> p B c", c=C),
                        in0=zin_sb[gb % 2][:, :nb_g * C].rearrange("p (B c) -> p B c", c=C),
                        in1=sdg_sb[:, g * GRP:g * GRP + nb_g].to_broadcast([P, nb_g, C]),
                        op=mybir.AluOpType.mult,
                    )
                    ve.wait_ge(s_mm, k * NB + blocks_thru(g))
                    pbase = ((g * GRP) % 8) * 512
                    ve.tensor_reduce(
                        out=agg_sb[:, :nb_g * C].rearrange("p (B c) -> p B c", c=C),
                        in_=psum[:, pbase:pbase + nb_g * 512].rearrange(
                            "p (B j c) -> p B c j", j=8, c=C),
                        axis=mybir.AxisListType.X, op=mybir.AluOpType.add,
                    ).then_inc(s_dve, 1)
                    ve.wait_ge(s_dve, dtick())
                    if gb >= 2:
                        ve.wait_ge(s_znw, 16 * (NG + gb - 1))
                    ve.tensor_tensor(
                        out=tmp_sb[:, :nb_g * C],
                        in0=tmp_sb[:, :nb_g * C],
                        in1=agg_sb[:, :nb_g * C],
                        op=mybir.AluOpType.add,
                    ).then_inc(s_dve, 1)
                    ve.wait_ge(s_dve, dtick())
                    ve.scalar_tensor_tensor(
                        out=znew_sb[gb % 2][:, :nb_g * C],
                        in0=tmp_sb[:, :nb_g * C],
                        scalar=1.0 - ALPHA,
                        in1=z0in_sb[gb % 2][:, :nb_g * C],
                        op0=mybir.AluOpType.mult,
                        op1=mybir.AluOpType.add,
                    ).then_inc(s_epi, 1)
            if DBG_NO_SOFTMAX:
                return
            for g in range(NG):
                nb_g = blocks_thru(g) - g * GRP
                ve.wait_ge(s_zin, 32 * K * NG + 16 * (g + 1))
                ve.tensor_reduce(
                    out=nm_sb[:, :nb_g],
                    in_=zin_sb[g % 2][:, :nb_g * C].rearrange("p (B c) -> p B c", c=C),
                    axis=mybir.AxisListType.X, op=mybir.AluOpType.max,
                    negate=True,
                ).then_inc(s_sm1, 1)
                ve.wait_ge(s_sm2, 2 * (g + 1))
                if g >= 2:
                    ve.wait_ge(s_out, 16 * (g - 1))
                for j in range(nb_g):
                    mm = ve.scalar_tensor_tensor(
                        out=sm_sb[g % 2][:, j * C:(j + 1) * C],
                        in0=zin_sb[g % 2][:, j * C:(j + 1) * C],
                        scalar=nm_sb[:, j:j + 1],
                        in1=lse_sb[:, j:j + 1].to_broadcast([P, C]),
                        op0=mybir.AluOpType.add,
                        op1=mybir.AluOpType.subtract,
                    )
                mm.then_inc(s_sm, 1)

        # ---------------- ACT ----------------
        @block.scalar
        def _(ac):
            ac.wait_ge(s_boot, 16 * NBOOT)
            for t in range(ntile):
                w = tw(t)
                hb = 0 if t % 2 == 0 else 4
                zb = 2 if t % 2 == 0 else 6
                ac.wait_ge(s_pe1, t + 1)
                if t >= 2:
                    ac.wait_ge(s_pe2, t - 1)
                for m in range(MH):
                    ac.activation(
                        out=ht_sb[t % 2][:, m, :w],
                        in_=psum[:, (hb + m) * 512:(hb + m) * 512 + w],
                        func=mybir.ActivationFunctionType.Relu,
                        bias=b1_sb[:, m:m + 1],
                    ).then_inc(s_act1, 1)
                nj = (w + P - 1) // P
                ac.wait_ge(s_fz, t + 1)
                if t >= 2:
                    ac.wait_ge(s_z0w, 16 * (t - 1))
                ac.activation(
                    out=z0o_sb[t % 2][:, :nj * C],
                    in_=zf_sb[t % 2][:, :nj * C],
                    func=mybir.ActivationFunctionType.Copy,
                    scale=ALPHA,
                ).then_inc(s_act3, 1)
            for k in range(K):
                for g in range(NG):
                    gb = k * NG + g
                    tlo, thi = int(gto[g]), int(gto[g + 1])
                    if gb >= NIB - 1:
                        ac.wait_ge(s_scale, gb - (NIB - 1))
                    ac.dma_start(idxb[gb % NIB][:, :(thi - tlo) * 8],
                                 idx_d[:, tlo * 8:thi * 8]).then_inc(s_idx, 16)
                    ac.dma_start(nrmb[gb % NIB][:, :thi - tlo],
                                 nrm_d[:, tlo:thi]).then_inc(s_idx, 16)
            if DBG_NO_SOFTMAX:
                return
            for g in range(NG):
                nb_g = blocks_thru(g) - g * GRP
                ac.wait_ge(s_sm1, g + 1)
                for j in range(nb_g):
                    ins_e = ac.activation(
                        out=esc_sb[:, j * C:(j + 1) * C],
                        in_=zin_sb[g % 2][:, j * C:(j + 1) * C],
                        func=mybir.ActivationFunctionType.Exp,
                        bias=nm_sb[:, j:j + 1],
                        accum_out=se_sb[:, j:j + 1],
                    )
                ins_e.then_inc(s_sm2, 1)
                ac.wait_ge(s_sm2, 2 * g + 1)
                ac.activation(
                    out=lse_sb[:, :nb_g],
                    in_=se_sb[:, :nb_g],
                    func=mybir.ActivationFunctionType.Ln,
                ).then_inc(s_sm2, 1)

    nc.compile()
    return nc


# ---------------------------------------------------------------------------
# execution: jit once, keep inputs device-resident
# ---------------------------------------------------------------------------

def _make_runner(nc, in_maps, n_cores):
    """Mirror concourse.bass2jax.run_bass_via_pjrt, but build the jitted
    callable once and keep the (concatenated, sharded) inputs resident on
    device, so repeated executions measure device time rather than host->
    device transfer of ~200MB over the axon tunnel."""
    import jax
    from concourse import bass2jax

    bass2jax.install_neuronx_cc_hook()
    partition_name = (nc.partition_id_tensor.name
                      if nc.partition_id_tensor else None)
    in_names, out_names, out_avals, zero_outs = [], [], [], []
    for alloc in nc.m.functions[0].allocations:
        if not isinstance(alloc, mybir.MemoryLocationSet):
            continue
        name = alloc.memorylocations[0].name
        if alloc.kind == "ExternalInput":
            if name != partition_name:
                in_names.append(name)
        elif alloc.kind == "ExternalOutput":
            out_names.append(name)
            shape = tuple(alloc.tensor_shape)
            dtype = mybir.dt.np(alloc.dtype)
            out_avals.append(jax.core.ShapedArray(shape, dtype))
            zero_outs.append(np.zeros(shape, dtype))
    n_params = len(in_names)
    all_in = list(in_names) + list(out_names)
    if partition_name is not None:
        all_in.append(partition_name)

    def _body(*args):
        operands = list(args)
        if partition_name is not None:
            operands.append(bass2jax.partition_id_tensor())
        outs = bass2jax._bass_exec_p.bind(
            *operands,
            out_avals=tuple(out_avals),
            in_names=tuple(all_in),
            out_names=tuple(out_names),
            lowering_input_output_aliases=(),
            sim_require_finite=True,
            sim_require_nnan=True,
            nc=nc,
        )
        return tuple(outs)

    devices = jax.devices()[:n_cores]
    mesh = bass2jax.Mesh(np.asarray(devices), ("core",))
    spec = bass2jax.PartitionSpec("core")
    concat_in = [
        np.concatenate([np.asarray(in_maps[c][nm]) for c in range(n_cores)],
                       axis=0)
        for nm in in_names]
    concat_zero = [np.zeros((n_cores * z.shape[0], *z.shape[1:]), z.dtype)
                   for z in zero_outs]
    sharding = jax.sharding.NamedSharding(mesh, spec)
    dev_in = [jax.device_put(a, sharding) for a in concat_in + concat_zero]
    jax.block_until_ready(dev_in)

    def _compile():
        return jax.jit(
            bass2jax.shard_map(
                _body, mesh=mesh,
                in_specs=(spec,) * (n_params + len(out_names)),
                out_specs=(spec,) * len(out_names),
                check_rep=False),
            keep_unused=True).lower(*dev_in).compile()

    jitted = bass2jax.fast_dispatch_compile(_compile)

    def dispatch():
        return jitted(*dev_in)

    def run():
        outs = jitted(*dev_in)
        jax.block_until_ready(outs)
        return outs
    run.dispatch = dispatch

    def fetch(outs):
        return [
            {nm: np.asarray(outs[i]).reshape(n_cores, *out_avals[i].shape)[c]
             for i, nm in enumerate(out_names)}
            for c in range(n_cores)]
    return run, fetch


# ---------------------------------------------------------------------------
# top level
# ---------------------------------------------------------------------------

def _make_in_maps(meta, inputs):
    D, C = meta["D"], meta["C"]
    MH = D // P
    b1p = np.zeros((P, MH), np.float32)
    b1 = np.asarray(inputs["b1"], np.float32)
    for m in range(MH):
        b1p[:, m] = b1[m * P:(m + 1) * P]
    b2b = np.tile(np.asarray(inputs["b2"], np.float32).reshape(1, C), (P, 1))
    idf32 = np.eye(P, dtype=np.float32)
    idbf = np.eye(P, dtype=np.float32).astype(ml_dtypes.bfloat16)

    in_maps = []
    for c in range(NCORES):
        cd = meta["core_data"][c]
        in_maps.append({
            "xT": np.asarray(cd["xT"]),
            "W1p": np.asarray(meta["W1p"]),
            "W2": np.asarray(meta["W2"]),
            "b1p": b1p, "b2b": b2b,
            "idf32": idf32, "idbf": np.asarray(idbf),
            "sdg": cd["s_sb"],
            "idxs": cd["idx_all"],
            "nrms": cd["norm_all"],
        })
    return in_maps


def _run(inputs, kiter):
    meta = _preprocess(
        inputs["x"], inputs["edge_index"], inputs["edge_weight_train"],
        inputs["x_weight"], inputs["W1"], inputs["b1"], inputs["W2"],
        inputs["b2"], kiter,
    )
    nc = _build(meta)
    in_maps = _make_in_maps(meta, inputs)

    meta["_nc"] = nc
    meta["_in_maps"] = in_maps
    run, fetch = _make_runner(nc, in_maps, NCORES)
    meta["_run_fn"] = run
    meta["_fetch_fn"] = fetch
    res_outs = run()
    results = fetch(res_outs)

    N, C = meta["N"], meta["C"]
    SHP = meta["SHP"]
    perm = meta["perm"]
    out_full = np.empty((N, C), np.float32)
    for c in range(NCORES):
        o = results[c]["out"]
        rows = np.arange(SHP)
        nodes = perm[c * SHP + rows]
        v = nodes >= 0
        out_full[nodes[v]] = o[rows[v]]
    return out_full, meta, results


def kernel(**inputs):
    out, _, _ = _run(inputs, KITER_DEFAULT)
    return out

